# revision 10
# baseline (speedup 1.0000x reference)
"""Trainium2 Bass kernel for nn_CDB_34333968564293 (dense_cnn).

out = sum_t unfold(x)_t * kernel_t + x
where kernel = reshape(conv1x1(conv3x3(lrelu(conv3x3(x+y)))))

Sharding: pure data parallel over 8 cores: core c -> batch c//2, image
row-half c%2 (rows [128*(c%2), 128*(c%2)+128) of 256). Host pre-slices a
4-row halo (zero padded at global image edges) so the device program is
identical on every core.

Per-core program (streaming 16 super-chunks of 8 output rows):
  s = x + y                                  (DVE, float32r rounded)
  conv1: 9 taps as K=64 accumulating matmuls (float32r), LeakyReLU evac on ACT
  conv2: same, plain copy evac on ACT
  conv3 (1x1, 64->576): 5 matmuls of M=128 (two tap-blocks stacked on the
         PSUM partition halves; w3 output permutation is folded into the
         weight layout on the host)
  elementwise: per-tap multiply against shifted windows of x (DVE, PSUM
         operand), tap-sum tree on DVE/GPSIMD with one cross-partition fold
         via gpsimd accumulate-DMA, then +x, DMA out.
"""

import numpy as np
from contextlib import ExitStack

import concourse.bacc as bacc
import concourse.tile as tile
import concourse.mybir as mybir
from concourse.bass_utils import run_bass_kernel_spmd

F32 = mybir.dt.float32
F32R = mybir.dt.float32r
BF16 = mybir.dt.bfloat16

C = 64          # channels
H = 256
W = 256
B = 4
NCORES = 8
RSH = 128       # rows per core shard
R = 8           # output rows per super-chunk
NSUP = RSH // R # 16
WP = W + 2      # padded row pitch

# tap order t = di*3 + dj; conv3 blocks pair taps (top partitions 0-63,
# bottom 64-127). Bottom member = top member + 3 (one row down) for blocks
# 0-2, matching the +1-row shifted copy of x held on partitions 64-127.
# Block 3 pairs (2,0)&(2,1) (handled with two half ops), block 4 is (2,2).
CONV3_BLOCKS = [(0, 3), (1, 4), (2, 5), (6, 7), (8, None)]

_CACHE = {}


def _build_nc():
    nc = bacc.Bacc("TRN2", target_bir_lowering=False, debug=False,
                   num_devices=NCORES)
    xh = nc.dram_tensor("xh", [C, RSH + 4, W], F32, kind="ExternalInput")
    yh = nc.dram_tensor("yh", [C, RSH + 4, W], F32, kind="ExternalInput")
    w1t = nc.dram_tensor("w1t", [C, 9, C], F32, kind="ExternalInput")
    w2t = nc.dram_tensor("w2t", [C, 9, C], F32, kind="ExternalInput")
    w3t = nc.dram_tensor("w3t", [C, 5, 128], F32, kind="ExternalInput")
    # host-computed k1 boundary rows (shard rows -1 and 128); zero when the
    # row falls outside the image (conv SAME pads the intermediate with 0)
    k1b = nc.dram_tensor("k1b", [C, 2, W], F32, kind="ExternalInput")
    out_d = nc.dram_tensor("out", [C, RSH, W], F32, kind="ExternalOutput")

    with tile.TileContext(nc) as tc:
        with ExitStack() as ctx:
            _kernel_body(ctx, tc, nc, xh, yh, w1t, w2t, w3t, k1b, out_d)
    nc.compile()
    return nc


def _kernel_body(ctx, tc, nc, xh, yh, w1t, w2t, w3t, k1b, out_d):
    const = ctx.enter_context(tc.tile_pool(name="const", bufs=1))
    stage = ctx.enter_context(tc.tile_pool(name="stage", bufs=2))
    prp = ctx.enter_context(tc.tile_pool(name="prp", bufs=2))
    ps1 = ctx.enter_context(tc.tile_pool(name="ps1", bufs=2, space="PSUM"))
    ps2 = ctx.enter_context(tc.tile_pool(name="ps2", bufs=1, space="PSUM"))
    ps3 = ctx.enter_context(tc.tile_pool(name="ps3", bufs=1, space="PSUM"))

    # --- weights: load once, round to f32r via DVE copy ---
    w1s = const.tile([C, 9, C], F32)
    w2s = const.tile([C, 9, C], F32)
    w3s = const.tile([C, 5, 128], F32)
    nc.sync.dma_start(out=w1s[:], in_=w1t.ap())
    nc.sync.dma_start(out=w2s[:], in_=w2t.ap())
    nc.sync.dma_start(out=w3s[:], in_=w3t.ap())
    w1r = const.tile([C, 9, C], F32R)
    w2r = const.tile([C, 9, C], F32R)
    w3r = const.tile([C, 5, 128], F32R)
    nc.vector.tensor_copy(w1r[:], w1s[:])
    nc.vector.tensor_copy(w2r[:], w2s[:])
    nc.vector.tensor_copy(w3r[:], w3s[:])

    for it in range(NSUP):
        base = it * R  # xh row of first halo row for this super-chunk

        # --- load x, y (12 halo rows), compute s = x + y ---
        xt = stage.tile([C, R + 4, WP], F32, tag="xt")
        yt = stage.tile([C, R + 4, WP], F32, tag="yt")
        nc.vector.memset(xt[:, :, 0:WP:W + 1], 0.0)   # pad cols 0, 257
        nc.vector.memset(yt[:, :, 0:WP:W + 1], 0.0)
        nc.sync.dma_start(out=xt[:, :, 1:W + 1], in_=xh.ap()[:, base:base + R + 4, :])
        nc.sync.dma_start(out=yt[:, :, 1:W + 1], in_=yh.ap()[:, base:base + R + 4, :])
        s = stage.tile([C, R + 4, WP], F32R, tag="s")
        nc.vector.tensor_add(s[:], xt[:], yt[:])

        # --- x window stack for the elementwise stage ---
        # top (partitions 0-63): xh rows [base+1, base+12)   (11 rows)
        # bottom (64-127):       xh rows [base+2, base+12)   (10 rows, +1 row)
        xk = stage.tile([128, R + 3, WP], F32, tag="xk")
        nc.vector.memset(xk[:, :, 0:WP:W + 1], 0.0)
        nc.vector.memset(xk[64:128, R + 2, :], 0.0)  # unused last bottom row
        nc.sync.dma_start(out=xk[0:64, 0:R + 3, 1:W + 1],
                          in_=xh.ap()[:, base + 1:base + R + 4, :])
        nc.sync.dma_start(out=xk[64:128, 0:R + 2, 1:W + 1],
                          in_=xh.ap()[:, base + 2:base + R + 4, :])

        # --- conv1: k1 rows = xh rows [base+1, base+11) (10 rows) ---
        k1 = stage.tile([C, R + 2, WP], F32R, tag="k1")
        nc.vector.memset(k1[:].bitcast(F32)[:, :, 0:WP:W + 1], 0.0)
        for c1 in range(R // 2 + 1):  # 5 chunks of 2 rows
            pc = ps1.tile([C, 2, W], F32, tag="pc1")
            for t in range(9):
                di, dj = t // 3, t % 3
                nc.tensor.matmul(
                    pc[:], w1r[:, t, :],
                    s[:, 2 * c1 + di:2 * c1 + di + 2, dj:dj + W],
                    start=(t == 0), stop=(t == 8))
            nc.scalar.activation(
                k1[:, 2 * c1:2 * c1 + 2, 1:W + 1], pc[:],
                mybir.ActivationFunctionType.Lrelu, alpha=0.01)
        # overwrite the out-of-shard k1 boundary row with host-supplied data
        if it == 0 or it == NSUP - 1:
            k1bs = stage.tile([C, 1, W], F32, tag="k1bs", name="k1bs")
            row = 0 if it == 0 else 1
            nc.sync.dma_start(out=k1bs[:], in_=k1b.ap()[:, row:row + 1, :])
            dst_r = 0 if it == 0 else R + 1
            nc.vector.tensor_copy(k1[:, dst_r:dst_r + 1, 1:W + 1], k1bs[:])

        # --- conv2: k2 rows = out rows [base+2, base+10) in xh coords ---
        k2 = stage.tile([C, R, WP], F32R, tag="k2")
        nc.vector.memset(k2[:].bitcast(F32)[:, :, 0:WP:W + 1], 0.0)
        for c2 in range(R // 2):  # 4 chunks
            pc = ps2.tile([C, 2, W], F32, tag="pc2")
            for t in range(9):
                di, dj = t // 3, t % 3
                nc.tensor.matmul(
                    pc[:], w2r[:, t, :],
                    k1[:, 2 * c2 + di:2 * c2 + di + 2, dj:dj + W],
                    start=(t == 0), stop=(t == 8))
            nc.scalar.activation(
                k2[:, 2 * c2:2 * c2 + 2, 1:W + 1], pc[:],
                mybir.ActivationFunctionType.Copy)

        # --- conv3 + elementwise, per 2-row out chunk ---
        osb = stage.tile([C, R, W], F32, tag="osb")
        for c3 in range(R // 2):
            pbs = []
            for bI in range(5):
                pb = ps3.tile([128, 2, W], F32, tag=f"pb{bI}", name=f"pb{bI}")
                nc.tensor.matmul(pb[:], w3r[:, bI, :],
                                 k2[:, 2 * c3:2 * c3 + 2, 1:W + 1],
                                 start=True, stop=True)
                pbs.append(pb)

            # products: tap t=(di,dj): top reads xk rows [2c3+di, +2) cols
            # [dj, dj+W); bottom (tap di+1 built in) same AP.
            pr = [prp.tile([128, 2, W], F32, tag=f"pr{i}", name=f"pr{i}")
                  for i in range(4)]
            pr5 = prp.tile([64, 2, W], F32, tag="pr5")
            for i, colj in enumerate((0, 1, 2)):  # blocks {t,t+3}, t=0,1,2
                nc.vector.tensor_mul(
                    pr[i][:], pbs[i][:],
                    xk[:, 2 * c3:2 * c3 + 2, colj:colj + W])
            # block 3: taps (2,0) top / (2,1) bottom
            nc.vector.tensor_mul(
                pr[3][0:64], pbs[3][0:64],
                xk[0:64, 2 * c3 + 2:2 * c3 + 4, 0:W])
            nc.vector.tensor_mul(
                pr[3][64:128], pbs[3][64:128],
                xk[64:128, 2 * c3 + 1:2 * c3 + 3, 1:W + 1])
            # block 4: tap (2,2) top only
            nc.vector.tensor_mul(
                pr5[:], pbs[4][0:64],
                xk[0:64, 2 * c3 + 2:2 * c3 + 4, 2:W + 2])

            a1 = prp.tile([128, 2, W], F32, tag="a1")
            a2 = prp.tile([128, 2, W], F32, tag="a2")
            nc.vector.tensor_add(a1[:], pr[0][:], pr[1][:])
            nc.gpsimd.tensor_add(a2[:], pr[2][:], pr[3][:])
            a3 = prp.tile([128, 2, W], F32, tag="a3")
            nc.vector.tensor_add(a3[:], a1[:], a2[:])
            # fold bottom half onto top half (cross-partition)
            nc.gpsimd.dma_start(out=a3[0:64], in_=a3[64:128],
                                accum_op=mybir.AluOpType.add)
            a4 = prp.tile([64, 2, W], F32, tag="a4")
            nc.vector.tensor_add(a4[:], a3[0:64], pr5[:])
            # + x (center tap window of top copy)
            nc.vector.tensor_add(
                osb[:, 2 * c3:2 * c3 + 2, :], a4[:],
                xk[0:64, 2 * c3 + 1:2 * c3 + 3, 1:W + 1])

        nc.sync.dma_start(out=out_d.ap()[:, base:base + R, :], in_=osb[:])


def _prep_weights(w1, w2, w3):
    # lhsT layouts: [ci(K), tap, co(M)]
    w1t = np.ascontiguousarray(w1.reshape(C, C, 9).transpose(1, 2, 0))
    w2t = np.ascontiguousarray(w2.reshape(C, C, 9).transpose(1, 2, 0))
    # conv3: w3[(co*9+t), e] -> blocks [e(K), block, 128]
    w3m = w3.reshape(C * 9, C)  # [co*9+t, e]
    w3t = np.zeros((C, 5, 128), np.float32)
    for bI, (t_top, t_bot) in enumerate(CONV3_BLOCKS):
        for co in range(C):
            w3t[:, bI, co] = w3m[co * 9 + t_top, :]
            if t_bot is not None:
                w3t[:, bI, 64 + co] = w3m[co * 9 + t_bot, :]
    return w1t, w2t, w3t


def _k1_row(x, y, w1, b, g):
    """Host conv1+lrelu at global row g (SAME padding); zeros if outside."""
    if g < 0 or g >= H:
        return np.zeros((C, W), np.float32)
    s = np.zeros((C, 3, W + 2), np.float32)
    lo, hi = max(g - 1, 0), min(g + 2, H)
    s[:, lo - (g - 1):hi - (g - 1), 1:W + 1] = (
        x[b, :, lo:hi, :] + y[b, :, lo:hi, :])
    acc = np.zeros((C, W), np.float32)
    for t in range(9):
        di, dj = t // 3, t % 3
        acc += np.einsum('kc,kw->cw', w1.reshape(C, C, 9)[:, :, t].T,
                         s[:, di, dj:dj + W])
    return np.where(acc > 0, acc, 0.01 * acc).astype(np.float32)


def _shard_inputs(x, y, w1, w2, w3):
    w1t, w2t, w3t = _prep_weights(w1, w2, w3)
    in_maps = []
    for c in range(NCORES):
        b, half = c // 2, c % 2
        r0 = half * RSH
        xp = np.zeros((C, RSH + 4, W), np.float32)
        yp = np.zeros((C, RSH + 4, W), np.float32)
        lo, hi = max(r0 - 2, 0), min(r0 + RSH + 2, H)
        xp[:, lo - (r0 - 2):hi - (r0 - 2), :] = x[b, :, lo:hi, :]
        yp[:, lo - (r0 - 2):hi - (r0 - 2), :] = y[b, :, lo:hi, :]
        k1b = np.stack([_k1_row(x, y, w1, b, r0 - 1),
                        _k1_row(x, y, w1, b, r0 + RSH)])
        k1b = np.ascontiguousarray(k1b.transpose(1, 0, 2))  # [C, 2, W]
        in_maps.append({"xh": xp, "yh": yp, "w1t": w1t, "w2t": w2t,
                        "w3t": w3t, "k1b": k1b})
    return in_maps


def kernel(x, y, w1, w2, w3):
    x = np.asarray(x, np.float32)
    y = np.asarray(y, np.float32)
    if "nc" not in _CACHE:
        _CACHE["nc"] = _build_nc()
    nc = _CACHE["nc"]
    in_maps = _shard_inputs(x, y, np.asarray(w1, np.float32),
                            np.asarray(w2, np.float32),
                            np.asarray(w3, np.float32))
    res = run_bass_kernel_spmd(nc, in_maps, core_ids=list(range(NCORES)))
    out = np.empty((B, C, H, W), np.float32)
    for c in range(NCORES):
        b, half = c // 2, c % 2
        out[b, :, half * RSH:half * RSH + RSH, :] = res.results[c]["out"]
    return out


# revision 11
# speedup vs baseline: 1.0863x; 1.0863x over previous
"""Trainium2 Bass kernel for nn_CDB_34333968564293 (dense_cnn).

out = sum_t unfold(x)_t * kernel_t + x
where kernel = reshape(conv1x1(conv3x3(lrelu(conv3x3(x+y)))))

Sharding: pure data parallel over 8 cores: core c -> batch c//2, image
row-half c%2. Host pre-slices a halo (zero padded at global image edges)
so the device program is identical on every core (SPMD).

Layout trick: x and y are loaded as 128-partition stacks (bottom half =
same image shifted +1 row). Then:
  * s = x+y on the full 128 partitions gives the conv input pair-stacked
    for free -> conv3x3 = 3 K=128 matmuls (tap rows 0&1) + 3 K=64 (row 2).
  * conv1 uses M=128 weights with duplicated output columns, so its PSUM
    holds two copies; two ScalarE Lrelu evacuations build the same
    pair-stacked layout for k1 (bottom shifted one row).
  * the elementwise stage reads tap windows of x directly as views into
    the X stack (tap di and di+1 in one [128,...] op).
Matmuls run in float32r (~2e-4 rel err, full PE speed at N>=256).
"""

import numpy as np
from contextlib import ExitStack

import concourse.bacc as bacc
import concourse.tile as tile
import concourse.mybir as mybir
from concourse.bass_utils import run_bass_kernel_spmd

F32 = mybir.dt.float32
F32R = mybir.dt.float32r
BF16 = mybir.dt.bfloat16
ADD = mybir.AluOpType.add

C = 64
H = 256
W = 256
B = 4
NCORES = 8
RSH = 128        # rows per core shard
R = 8            # output rows per super-chunk
NSUP = RSH // R  # 16
WP = W + 2       # padded row pitch
HALO = RSH + 5   # xh rows: [r0-2, r0+131)

_CACHE = {}


def _build_nc():
    nc = bacc.Bacc("TRN2", target_bir_lowering=False, debug=False,
                   num_devices=NCORES)
    xh = nc.dram_tensor("xh", [C, HALO, W], F32, kind="ExternalInput")
    yh = nc.dram_tensor("yh", [C, HALO, W], F32, kind="ExternalInput")
    # paired conv weights: [128, 3, 64] rows = taps (0,j)(top)/(1,j)(bot);
    # w1p has duplicated output columns -> [128, 3, 128]
    w1p = nc.dram_tensor("w1p", [128, 3, 128], F32, kind="ExternalInput")
    w1q = nc.dram_tensor("w1q", [C, 3, 128], F32, kind="ExternalInput")
    w2p = nc.dram_tensor("w2p", [128, 3, C], F32, kind="ExternalInput")
    w2q = nc.dram_tensor("w2q", [C, 3, C], F32, kind="ExternalInput")
    w3t = nc.dram_tensor("w3t", [C, 5, 128], F32, kind="ExternalInput")
    # host-computed k1 boundary rows (shard rows -1 and 128)
    k1b = nc.dram_tensor("k1b", [C, 2, W], F32, kind="ExternalInput")
    out_d = nc.dram_tensor("out", [C, RSH, W], F32, kind="ExternalOutput")

    with tile.TileContext(nc) as tc:
        with ExitStack() as ctx:
            _body(ctx, tc, nc, xh, yh, w1p, w1q, w2p, w2q, w3t, k1b, out_d)
    nc.compile()
    return nc


def _body(ctx, tc, nc, xh, yh, w1p, w1q, w2p, w2q, w3t, k1b, out_d):
    const = ctx.enter_context(tc.tile_pool(name="const", bufs=1))
    stage = ctx.enter_context(tc.tile_pool(name="stage", bufs=2))
    prp = ctx.enter_context(tc.tile_pool(name="prp", bufs=2))
    ps1 = ctx.enter_context(tc.tile_pool(name="ps1", bufs=2, space="PSUM"))
    ps2 = ctx.enter_context(tc.tile_pool(name="ps2", bufs=1, space="PSUM"))
    ps3 = ctx.enter_context(tc.tile_pool(name="ps3", bufs=1, space="PSUM"))

    # --- weights: load once, round to f32r ---
    w1ps = const.tile([128, 3, 128], F32)
    w1qs = const.tile([C, 3, 128], F32)
    w2ps = const.tile([128, 3, C], F32)
    w2qs = const.tile([C, 3, C], F32)
    w3s = const.tile([C, 5, 128], F32)
    for tdst, tsrc in ((w1ps, w1p), (w1qs, w1q), (w2ps, w2p), (w2qs, w2q),
                       (w3s, w3t)):
        nc.sync.dma_start(out=tdst[:], in_=tsrc.ap())
    w1pr = const.tile([128, 3, 128], F32R)
    w1qr = const.tile([C, 3, 128], F32R)
    w2pr = const.tile([128, 3, C], F32R)
    w2qr = const.tile([C, 3, C], F32R)
    w3r = const.tile([C, 5, 128], F32R)
    nc.vector.tensor_copy(w1pr[:], w1ps[:])
    nc.vector.tensor_copy(w1qr[:], w1qs[:])
    nc.vector.tensor_copy(w2pr[:], w2ps[:])
    nc.vector.tensor_copy(w2qr[:], w2qs[:])
    nc.vector.tensor_copy(w3r[:], w3s[:])

    for it in range(NSUP):
        base = it * R

        # --- X/Y stacks: top = xh rows [base, base+12); bottom = +1 row ---
        X = stage.tile([128, R + 4, WP], F32, tag="X")
        Y = stage.tile([128, R + 4, WP], F32, tag="Y")
        nc.vector.memset(X[:, :, 0:WP:W + 1], 0.0)
        nc.vector.memset(Y[:, :, 0:WP:W + 1], 0.0)
        nc.sync.dma_start(out=X[0:64, :, 1:W + 1],
                          in_=xh.ap()[:, base:base + R + 4, :])
        nc.sync.dma_start(out=X[64:128, :, 1:W + 1],
                          in_=xh.ap()[:, base + 1:base + R + 5, :])
        nc.sync.dma_start(out=Y[0:64, :, 1:W + 1],
                          in_=yh.ap()[:, base:base + R + 4, :])
        nc.sync.dma_start(out=Y[64:128, :, 1:W + 1],
                          in_=yh.ap()[:, base + 1:base + R + 5, :])
        S = stage.tile([128, R + 4, WP], F32R, tag="S")
        nc.vector.tensor_add(S[:], X[:], Y[:])

        # xkB stack for conv3 block {(2,0),(2,1)}: top = x, bottom = x
        # shifted +1 col (rows xh [base+1, base+12))
        xkB = stage.tile([128, R + 3, WP], F32, tag="xkB")
        nc.vector.memset(xkB[:, :, 0:WP:W + 1], 0.0)
        nc.vector.memset(xkB[64:128, :, W:W + 2], 0.0)
        nc.sync.dma_start(out=xkB[0:64, :, 1:W + 1],
                          in_=xh.ap()[:, base + 1:base + R + 4, :])
        nc.sync.dma_start(out=xkB[64:128, :, 0:W],
                          in_=xh.ap()[:, base + 1:base + R + 4, :])

        # --- conv1 -> k1 stack [128, R+2, WP]:
        #     top rows [0,R+2) = k1 global rows base-1+r
        #     bottom rows [0,R+1): bottom[r] = k1[r+1]
        k1 = stage.tile([128, R + 2, WP], F32R, tag="k1")
        nc.vector.memset(k1[:].bitcast(F32)[:, :, 0:WP:W + 1], 0.0)
        for c1 in range(R // 2 + 1):
            pc = ps1.tile([128, 2, W], F32, tag="pc1")
            for j in range(3):
                nc.tensor.matmul(pc[:], w1pr[:, j, :],
                                 S[:, 2 * c1:2 * c1 + 2, j:j + W],
                                 start=(j == 0), stop=False)
            for j in range(3):
                nc.tensor.matmul(pc[:], w1qr[:, j, :],
                                 S[0:64, 2 * c1 + 2:2 * c1 + 4, j:j + W],
                                 start=False, stop=(j == 2))
            nc.scalar.activation(
                k1[0:64, 2 * c1:2 * c1 + 2, 1:W + 1], pc[0:64],
                mybir.ActivationFunctionType.Lrelu, alpha=0.01)
            if c1 == 0:
                nc.scalar.activation(
                    k1[64:128, 0:1, 1:W + 1], pc[64:128, 1:2, :],
                    mybir.ActivationFunctionType.Lrelu, alpha=0.01)
            else:
                nc.scalar.activation(
                    k1[64:128, 2 * c1 - 1:2 * c1 + 1, 1:W + 1], pc[64:128],
                    mybir.ActivationFunctionType.Lrelu, alpha=0.01)

        # boundary k1 row overwrite (host-supplied; SPMD-safe)
        if it == 0 or it == NSUP - 1:
            k1bs = stage.tile([C, 1, W], F32, tag="k1bs", name="k1bs")
            row = 0 if it == 0 else 1
            nc.sync.dma_start(out=k1bs[:], in_=k1b.ap()[:, row:row + 1, :])
            if it == 0:
                nc.vector.tensor_copy(k1[0:64, 0:1, 1:W + 1], k1bs[:])
            else:
                nc.vector.tensor_copy(k1[0:64, R + 1:R + 2, 1:W + 1], k1bs[:])
                nc.vector.tensor_copy(k1[64:128, R:R + 1, 1:W + 1], k1bs[:])

        # --- conv2 -> k2 [64, R, WP] (k2 rows = out rows [base, base+8)) ---
        k2 = stage.tile([C, R, WP], F32R, tag="k2")
        nc.vector.memset(k2[:].bitcast(F32)[:, :, 0:WP:W + 1], 0.0)
        for c2 in range(R // 2):
            pc = ps2.tile([C, 2, W], F32, tag="pc2")
            for j in range(3):
                nc.tensor.matmul(pc[:], w2pr[:, j, :],
                                 k1[:, 2 * c2:2 * c2 + 2, j:j + W],
                                 start=(j == 0), stop=False)
            for j in range(3):
                nc.tensor.matmul(pc[:], w2qr[:, j, :],
                                 k1[0:64, 2 * c2 + 2:2 * c2 + 4, j:j + W],
                                 start=False, stop=(j == 2))
            nc.scalar.activation(k2[:, 2 * c2:2 * c2 + 2, 1:W + 1], pc[:],
                                 mybir.ActivationFunctionType.Copy)

        # --- conv3 + elementwise per 2-row chunk ---
        # out rows global [base+2c3, +2); X-top row r = xh row base+r;
        # window for tap (di,dj) at X-top rows [2c3+1+di, +2) cols [dj,dj+W);
        # X-bottom supplies tap (di+1,dj) at the same AP.
        for c3 in range(R // 2):
            pbs = []
            for bI in range(5):
                mm = 128 if bI < 4 else 64
                pb = ps3.tile([mm, 2, W], F32, tag=f"pb{bI}", name=f"pb{bI}")
                nc.tensor.matmul(pb[:], w3r[:, bI, 0:mm],
                                 k2[:, 2 * c3:2 * c3 + 2, 1:W + 1],
                                 start=True, stop=True)
                pbs.append(pb)

            pr = [prp.tile([128, 2, W], BF16, tag=f"pr{i}", name=f"pr{i}")
                  for i in range(4)]
            pr5 = prp.tile([C, 2, W], BF16, tag="pr5", name="pr5")
            # blocks {(0,j),(1,j)}: one [128] op each
            for j in range(3):
                nc.vector.tensor_mul(pr[j][:], pbs[j][:],
                                     X[:, 2 * c3 + 1:2 * c3 + 3, j:j + W])
            # block {(2,0),(2,1)} via xkB (bottom = +1 col)
            nc.vector.tensor_mul(pr[3][:], pbs[3][:],
                                 xkB[:, 2 * c3 + 2:2 * c3 + 4, 0:W])
            # block {(2,2)} top only
            nc.vector.tensor_mul(pr5[:], pbs[4][:],
                                 X[0:64, 2 * c3 + 3:2 * c3 + 5, 2:W + 2])

            a1 = prp.tile([128, 2, W], BF16, tag="a1", name="a1")
            a2 = prp.tile([128, 2, W], BF16, tag="a2", name="a2")
            nc.gpsimd.tensor_add(a1[:], pr[0][:], pr[1][:])
            nc.gpsimd.tensor_add(a2[:], pr[2][:], pr[3][:])
            a3 = prp.tile([128, 2, W], BF16, tag="a3", name="a3")
            nc.vector.tensor_add(a3[:], a1[:], a2[:])
            # top half: + pr5 ; bottom half: + x itself (residual, fp32)
            a5 = prp.tile([128, 2, W], F32, tag="a5", name="a5")
            nc.gpsimd.tensor_add(a5[0:64], a3[0:64], pr5[:])
            nc.vector.tensor_add(a5[64:128], a3[64:128],
                                 X[64:128, 2 * c3 + 1:2 * c3 + 3, 1:W + 1])
            # fold bottom onto top (cross-partition) then store
            nc.gpsimd.dma_start(out=a5[0:64], in_=a5[64:128], accum_op=ADD)
            nc.sync.dma_start(
                out=out_d.ap()[:, base + 2 * c3:base + 2 * c3 + 2, :],
                in_=a5[0:64])


def _prep_weights(w1, w2, w3):
    w1m = w1.reshape(C, C, 9)  # [co, ci, t]
    w2m = w2.reshape(C, C, 9)
    # paired stacks: rows 0-63 tap (0,j), 64-127 tap (1,j)
    w1p = np.zeros((128, 3, 128), np.float32)
    w1q = np.zeros((C, 3, 128), np.float32)
    w2p = np.zeros((128, 3, C), np.float32)
    w2q = np.zeros((C, 3, C), np.float32)
    for j in range(3):
        w1p[0:64, j, 0:64] = w1m[:, :, 0 + j].T
        w1p[64:128, j, 0:64] = w1m[:, :, 3 + j].T
        w1p[:, j, 64:128] = w1p[:, j, 0:64]      # duplicated out columns
        w1q[:, j, 0:64] = w1m[:, :, 6 + j].T
        w1q[:, j, 64:128] = w1q[:, j, 0:64]
        w2p[0:64, j, :] = w2m[:, :, 0 + j].T
        w2p[64:128, j, :] = w2m[:, :, 3 + j].T
        w2q[:, j, :] = w2m[:, :, 6 + j].T
    # conv3 blocks: pairs {t,t+3} t=0,1,2 then {6,7}, {8}
    w3m = w3.reshape(C * 9, C)  # [co*9+t, e]
    w3t = np.zeros((C, 5, 128), np.float32)
    blocks = [(0, 3), (1, 4), (2, 5), (6, 7), (8, None)]
    for bI, (t_top, t_bot) in enumerate(blocks):
        for co in range(C):
            w3t[:, bI, co] = w3m[co * 9 + t_top, :]
            if t_bot is not None:
                w3t[:, bI, 64 + co] = w3m[co * 9 + t_bot, :]
    return w1p, w1q, w2p, w2q, w3t


def _k1_row(x, y, w1, b, g):
    """Host conv1+lrelu at global row g (SAME padding); zeros if outside."""
    if g < 0 or g >= H:
        return np.zeros((C, W), np.float32)
    s = np.zeros((C, 3, W + 2), np.float32)
    lo, hi = max(g - 1, 0), min(g + 2, H)
    s[:, lo - (g - 1):hi - (g - 1), 1:W + 1] = (
        x[b, :, lo:hi, :] + y[b, :, lo:hi, :])
    acc = np.zeros((C, W), np.float32)
    for t in range(9):
        di, dj = t // 3, t % 3
        acc += np.einsum('kc,kw->cw', w1.reshape(C, C, 9)[:, :, t].T,
                         s[:, di, dj:dj + W])
    return np.where(acc > 0, acc, 0.01 * acc).astype(np.float32)


def _shard_inputs(x, y, w1, w2, w3):
    w1p, w1q, w2p, w2q, w3t = _prep_weights(w1, w2, w3)
    in_maps = []
    for c in range(NCORES):
        b, half = c // 2, c % 2
        r0 = half * RSH
        xp = np.zeros((C, HALO, W), np.float32)
        yp = np.zeros((C, HALO, W), np.float32)
        lo, hi = max(r0 - 2, 0), min(r0 + RSH + 3, H)
        xp[:, lo - (r0 - 2):hi - (r0 - 2), :] = x[b, :, lo:hi, :]
        yp[:, lo - (r0 - 2):hi - (r0 - 2), :] = y[b, :, lo:hi, :]
        k1bv = np.stack([_k1_row(x, y, w1, b, r0 - 1),
                         _k1_row(x, y, w1, b, r0 + RSH)])
        k1bv = np.ascontiguousarray(k1bv.transpose(1, 0, 2))
        in_maps.append({"xh": xp, "yh": yp, "w1p": w1p, "w1q": w1q,
                        "w2p": w2p, "w2q": w2q, "w3t": w3t, "k1b": k1bv})
    return in_maps


def kernel(x, y, w1, w2, w3):
    x = np.asarray(x, np.float32)
    y = np.asarray(y, np.float32)
    if "nc" not in _CACHE:
        _CACHE["nc"] = _build_nc()
    nc = _CACHE["nc"]
    in_maps = _shard_inputs(x, y, np.asarray(w1, np.float32),
                            np.asarray(w2, np.float32),
                            np.asarray(w3, np.float32))
    res = run_bass_kernel_spmd(nc, in_maps, core_ids=list(range(NCORES)))
    out = np.empty((B, C, H, W), np.float32)
    for c in range(NCORES):
        b, half = c // 2, c % 2
        out[b, :, half * RSH:half * RSH + RSH, :] = res.results[c]["out"]
    return out
